# revision 27
# baseline (speedup 1.0000x reference)
# GNN edge-apply MLP kernel for Trainium2 (Bass/Tile), 8-core SPMD.
#
# reference semantics:
#   feat = concat(node_feats[src], node_feats[dst], axis=1)      # [E, 2048]
#   h    = relu(feat @ W1 + b1)                                  # [E, 1024]
#   out  = h @ W2 + b2                                           # [E, 1024]
#
# Sharding: edges split evenly across 8 cores (8192 each); node table and
# weights replicated.  All matmul operands are bf16 (host-cast): same PE rate
# as fp32r (1 row/cycle) but half the DMA/DVE traffic, 1.0 instead of 1.5
# cycles/row transposes, and rel-err ~3e-3 (vs the 2e-2 gate).
#
# Dataflow (per 512-edge supertile, 16 per core): layer 1 is computed
# TRANSPOSED (out1T[h, e] = sum_k W1[k, hc].T @ featT[k, e]) so the relu
# output lands directly in the [h, e] layout that layer 2 needs as its
# stationary operand -- this kills the 8 h-transposes per 128-edge tile and
# the hT PSUM->SBUF copy of the old layout.  Relu+bias runs on the Scalar
# engine with per-partition bias (b1 rearranged so partition p of chunk hc
# holds b1[hc*128+p]), reading PSUM and writing bf16 SBUF in one pass.
#
# Per supertile:
#   - 8 indirect-DMA gathers (4 subtiles x src/dst), [128, 1024] bf16 rows
#   - 64 PE transposes -> featT chunks [128 f, 512 e] (staged via PSUM,
#     copied to SBUF by DVE in [128, 1024] chunks)
#   - L1: 8 h-chunks x 16 k-chunks matmuls (ap=512) -> psum [128 h, 512 e]
#   - ACT relu + per-partition b1, psum -> hT SBUF bf16
#   - L2: 4 e-chunks x 16 matmuls (ap=512) -> psum [128 e, 1024]
#   - DVE + b2 -> out tile f32 -> HWDGE store
import os
import sys

import numpy as np

for _p in ("/opt/trn_rl_repo",):
    if _p not in sys.path:
        sys.path.insert(0, _p)

N_NODES = 50000
D_NODE = 1024
D_HID = 1024
N_CORES = 8
E_TOTAL = 65536
E_CORE = E_TOTAL // N_CORES
P = 128


def build_nc(e_core=E_CORE, n_nodes=N_NODES):
    import concourse.bass as bass
    import concourse.mybir as mybir
    import concourse.tile as tile
    from concourse import bacc
    from concourse.masks import make_identity

    f32 = mybir.dt.float32
    bf16 = mybir.dt.bfloat16
    i32 = mybir.dt.int32

    T = e_core // P  # 64 subtiles of 128 edges
    SUP = 4  # subtiles per supertile
    S = T // SUP  # 16 supertiles
    ES = SUP * P  # 512 edges per supertile
    KD = (2 * D_NODE) // P  # 16 contraction chunks, layer 1
    KH = D_HID // P  # 8 contraction chunks, layer 2

    nc = bacc.Bacc(None, target_bir_lowering=False)

    nf = nc.dram_tensor("node_feats", [n_nodes, D_NODE], bf16, kind="ExternalInput")
    # W1/W2 arrive host-permuted as [128, K, 1024] (partition-major: entry
    # [p, k, :] = W[k*128+p, :]) so each weight load is one 32KB descriptor
    # per partition -- the hardware DGE queues are descriptor-dispatch-rate
    # limited (~60GB/s with 2KB descriptors), which used to stall layer 1 of
    # supertile 0 until ~50us.  b1 likewise arrives as [128, KH].
    w1 = nc.dram_tensor("W1", [P, KD, D_HID], bf16, kind="ExternalInput")
    w2 = nc.dram_tensor("W2", [P, KH, D_HID], bf16, kind="ExternalInput")
    b1 = nc.dram_tensor("b1", [P, KH], f32, kind="ExternalInput")
    b2 = nc.dram_tensor("b2", [D_HID], f32, kind="ExternalInput")
    src = nc.dram_tensor("src", [e_core], i32, kind="ExternalInput")
    dst = nc.dram_tensor("dst", [e_core], i32, kind="ExternalInput")
    out = nc.dram_tensor("out", [e_core, D_HID], f32, kind="ExternalOutput")

    nf_ap = nf.ap()
    # edge e of the core shard maps to (p, t) = (e // T, e % T): index loads
    # and output stores are contiguous per partition.
    out_v = out.ap().rearrange("(p t) h -> p t h", t=T)

    with tile.TileContext(nc) as tc:
        with (
            tc.tile_pool(name="const", bufs=1) as const_pool,
            tc.tile_pool(name="wpool", bufs=1) as wpool,
            tc.tile_pool(name="gather", bufs=2) as gather_pool,
            tc.tile_pool(name="featT", bufs=2) as featT_pool,
            tc.tile_pool(name="hT", bufs=2) as hT_pool,
            tc.tile_pool(name="outp", bufs=2) as out_pool,
            tc.tile_pool(name="pstf", bufs=2, space="PSUM") as psT_pool,
            tc.tile_pool(name="ps1", bufs=3, space="PSUM") as ps1_pool,
            tc.tile_pool(name="ps2", bufs=2, space="PSUM") as ps2_pool,
        ):
            # ---- constants / weights ----
            # zeros tile first: PE warmup transposes read it, and a single
            # gpsimd memset is ready ~1us before the identity construction.
            zeros_bf = const_pool.tile([P, P], bf16)
            nc.gpsimd.memset(zeros_bf[:], 0.0)
            ident_f32 = const_pool.tile([P, P], f32)
            ident = const_pool.tile([P, P], bf16)

            def build_ident():
                make_identity(nc, ident_f32[:])
                nc.vector.tensor_copy(ident[:], ident_f32[:])

            # src and dst indices side by side in one tile so a whole
            # supertile (4 subtiles x src+dst = 1024 rows) gathers with a
            # single SWDGE instruction (offset AP [128, 2, 4]) instead of 8 --
            # the ~1.1us/instruction descriptor-generation cost on the gpsimd
            # sequencer was the startup critical path.
            idx_all = const_pool.tile([P, 2, T], i32)
            nc.sync.dma_start(idx_all[:, 0, :], src.ap().rearrange("(p t) -> p t", t=T))
            nc.sync.dma_start(idx_all[:, 1, :], dst.ap().rearrange("(p t) -> p t", t=T))

            # W1 as [128, KD, 1024]: chunk k holds rows k*128..k*128+127 (f on
            # partitions -> natural lhsT for the transposed layer-1 matmuls).
            # W2 as [128, KH, 1024] likewise (h on partitions -> natural
            # moving operand for layer 2).  Interleaved 2:1 so W2 arrives
            # before supertile 0 reaches layer 2, and spread across three DGE
            # queues so descriptor generation isn't serialized behind one
            # sequencer at startup.
            # Strict priority: W1 first (it gates layer 1 of supertile 0;
            # the hardware DMA queues drain in order), split across both
            # HWDGE queues; then b1T (gates the first relu and with it the
            # ps1 pool), then W2 (needed ~15us later), then the 512KB b2
            # broadcast.
            w1_sb = wpool.tile([P, KD, D_HID], bf16)
            w2_sb = wpool.tile([P, KH, D_HID], bf16)
            nc.sync.dma_start(w1_sb[:, : KD // 2], w1.ap()[:, : KD // 2])
            nc.scalar.dma_start(w1_sb[:, KD // 2 :], w1.ap()[:, KD // 2 :])
            b1T = const_pool.tile([P, KH], f32)
            nc.sync.dma_start(b1T[:], b1.ap())
            nc.sync.dma_start(w2_sb[:, : KH // 2], w2.ap()[:, : KH // 2])
            nc.scalar.dma_start(w2_sb[:, KH // 2 :], w2.ap()[:, KH // 2 :])
            b2_bc = const_pool.tile([P, D_HID], f32)
            nc.scalar.dma_start(b2_bc[:], b2.ap()[None, :].to_broadcast([P, D_HID]))

            gsync = const_pool.tile([P, 1], bf16)

            def stage_G(s, after=None):
                """Indirect gathers for supertile s: gf[p, i*4+j, :] =
                node_feats[idx_all[p, i, 4s+j], :] (i = 0 src / 1 dst).
                All 4 src rows gather before the dst rows -- the transposes
                consume src chunks (pairs 0..3) first.

                `after`: featT tile whose last chunk must be written before
                these gathers start.  Engine queues run in program order, so
                this dummy gpsimd read is the only way to hold gathers back;
                used for supertiles 1-2, whose 2MB would otherwise compete
                with the W1 stream that gates L1(0) on startup HBM bandwidth.
                """
                if after is not None:
                    nc.gpsimd.tensor_copy(gsync[:], after[:, KD - 1, 0:1])
                gf = gather_pool.tile([P, 2 * SUP, D_NODE], bf16, tag="gf")
                for i in range(2):
                    for j in range(SUP):
                        t = SUP * s + j
                        nc.gpsimd.indirect_dma_start(
                            out=gf[:, i * SUP + j, :],
                            out_offset=None,
                            in_=nf_ap[:],
                            in_offset=bass.IndirectOffsetOnAxis(
                                ap=idx_all[:, i, t : t + 1], axis=0
                            ),
                        )
                return gf

            def warmup(n):
                """Dummy PE transposes of a zeros tile into scratch PSUM:
                keeps the Tensor-engine clock at full p-state while real
                operands are still in flight (a PE idle gap drops the clock
                to 1.2 GHz for the next ~3us of work)."""
                wps = psT_pool.tile([P, 2, ES], bf16, tag="psT")
                for i in range(n):
                    nc.tensor.transpose(wps[:, i % 2, 0:P], zeros_bf[:], zeros_bf[:])

            def stage_T(s, gf, fill=0):
                """PE transposes -> featT [128, KD, ES] bf16 in SBUF."""
                fT = featT_pool.tile([P, KD, ES], bf16, tag="featT")
                for pair in range(KD // 2):
                    if fill and pair == KD // 4:
                        warmup(fill)  # src gathers done, dst still landing
                    psT = psT_pool.tile([P, 2, ES], bf16, tag="psT")
                    for q in range(2):
                        k = 2 * pair + q
                        i, kk = (0, k) if k < KD // 2 else (1, k - KD // 2)
                        for j in range(SUP):
                            blk = gf[:, i * SUP + j, kk * P : (kk + 1) * P]
                            nc.tensor.transpose(
                                psT[:, q, j * P : (j + 1) * P], blk, ident[:]
                            )
                    nc.vector.tensor_copy(fT[:, 2 * pair : 2 * pair + 2, :], psT[:])
                return fT

            def stage_L1(s, fT):
                """Transposed layer 1 + fused relu/bias -> hT [128, KH, ES]."""
                hT = hT_pool.tile([P, KH, ES], bf16, tag="hT")
                for hc in range(KH):
                    ps1 = ps1_pool.tile([P, ES], f32, tag="ps1")
                    for k in range(KD):
                        nc.tensor.matmul(
                            ps1[:],
                            w1_sb[:, k, hc * P : (hc + 1) * P],
                            fT[:, k, :],
                            start=(k == 0),
                            stop=(k == KD - 1),
                        )
                    nc.scalar.activation(
                        hT[:, hc, :],
                        ps1[:],
                        mybir.ActivationFunctionType.Relu,
                        bias=b1T[:, hc : hc + 1],
                    )
                return hT

            def stage_L2(s, hT):
                """Layer 2 per 128-edge subtile, +b2; one batched store per
                supertile (partition p's 4 output rows are contiguous 16KB in
                DRAM, so the store is one descriptor per partition instead of
                4x128 4KB descriptors -- the store queue was within ~10%% of
                its descriptor-dispatch limit)."""
                o_sb = out_pool.tile([P, SUP, D_HID], f32, tag="osb")
                for ec in range(SUP):
                    for half in range(2):
                        ps2 = ps2_pool.tile([P, 512], f32, tag="ps2")
                        for k in range(KH):
                            nc.tensor.matmul(
                                ps2[:],
                                hT[:, k, ec * P : (ec + 1) * P],
                                w2_sb[:, k, half * 512 : (half + 1) * 512],
                                start=(k == 0),
                                stop=(k == KH - 1),
                            )
                        nc.vector.tensor_add(
                            o_sb[:, ec, half * 512 : (half + 1) * 512],
                            ps2[:],
                            b2_bc[:, half * 512 : (half + 1) * 512],
                        )
                    # store in 2-subtile chunks (8KB contiguous per partition);
                    # the last supertile stores per subtile so the kernel tail
                    # only waits on a 512KB transfer instead of 2MB.
                    step = 1 if s == S - 1 else 2
                    if s == S - 1 and ec == SUP - 1:
                        for half in range(2):
                            nc.scalar.dma_start(
                                out_v[:, SUP * s + ec, half * 512 : (half + 1) * 512],
                                o_sb[:, ec, half * 512 : (half + 1) * 512],
                            )
                    elif (ec + 1) % step == 0:
                        lo = ec + 1 - step
                        nc.scalar.dma_start(
                            out_v[:, SUP * s + lo : SUP * s + ec + 1, :],
                            o_sb[:, lo : ec + 1, :],
                        )

            # PE warmup: the Tensor engine clock ramps (0.65 -> 1.2 -> 2.4 GHz)
            # only after ~3us of continuous busy, and the first real transposes
            # can't start until the first gathers land (~13us in).  Keep the PE
            # busy from t~1us with dummy identity transposes into a scratch
            # PSUM tile so the clock is at full speed when real work arrives.
            # software pipeline: gathers run 2 supertiles ahead, transposes 1
            # ahead; PE stream per iteration is [L1(s) | T(s+1) | L2(s)] so the
            # last relu of s and the featT copies of s+1 hide under PE work.
            # warmup(200) spans the ~10us from the framework preamble until
            # supertile 0's src gathers land.  G(1) is issued only after T(0):
            # startup is HBM-bandwidth-bound, and G(1)'s 2MB of gathers would
            # otherwise compete with the W1 stream that gates L1(0) (T(1)
            # doesn't need G(1) until ~55us).
            gt = {0: stage_G(0)}
            build_ident()
            warmup(200)
            fTs = {0: stage_T(0, gt.pop(0), fill=70)}
            if S > 1:
                gt[1] = stage_G(1, after=fTs[0])
            for s in range(S):
                hT = stage_L1(s, fTs.pop(s))
                if s + 1 < S:
                    fTs[s + 1] = stage_T(s + 1, gt.pop(s + 1))
                if s + 2 < S:
                    gt[s + 2] = stage_G(
                        s + 2, after=fTs[s + 1] if s + 2 == 2 else None
                    )
                stage_L2(s, hT)

    nc.compile()
    return nc


LAST_RESULTS = None


def kernel(**inputs):
    global LAST_RESULTS
    import ml_dtypes
    from concourse.bass_utils import run_bass_kernel_spmd

    bf16 = ml_dtypes.bfloat16
    node_feats = np.ascontiguousarray(np.asarray(inputs["node_feats"]).astype(bf16))
    # weights/b1 pre-permuted to the partition-major DRAM layouts build_nc
    # declares (one large descriptor per partition on load)
    W1 = np.ascontiguousarray(
        np.asarray(inputs["W1"]).astype(bf16).reshape(16, 128, 1024).swapaxes(0, 1)
    )
    W2 = np.ascontiguousarray(
        np.asarray(inputs["W2"]).astype(bf16).reshape(8, 128, 1024).swapaxes(0, 1)
    )
    b1 = np.ascontiguousarray(
        np.asarray(inputs["b1"], np.float32).reshape(8, 128).T
    )
    b2 = np.ascontiguousarray(np.asarray(inputs["b2"], np.float32))
    src = np.ascontiguousarray(np.asarray(inputs["src"]).astype(np.int32))
    dst = np.ascontiguousarray(np.asarray(inputs["dst"]).astype(np.int32))

    nc = build_nc()

    in_maps = []
    for c in range(N_CORES):
        sl = slice(c * E_CORE, (c + 1) * E_CORE)
        in_maps.append(
            {
                "node_feats": node_feats,
                "W1": W1,
                "W2": W2,
                "b1": b1,
                "b2": b2,
                "src": src[sl],
                "dst": dst[sl],
            }
        )

    trace = bool(int(os.environ.get("KERNEL_TRACE", "0")))
    kw = {}
    if trace and bool(int(os.environ.get("KERNEL_TRACE_ALL", "0"))):
        kw["trace_cores"] = list(range(N_CORES))
    res = run_bass_kernel_spmd(
        nc, in_maps, core_ids=list(range(N_CORES)), trace=trace, **kw
    )
    LAST_RESULTS = res
    return np.concatenate([r["out"] for r in res.results], axis=0)


# revision 29
# speedup vs baseline: 1.0100x; 1.0100x over previous
# GNN edge-apply MLP kernel for Trainium2 (Bass/Tile), 8-core SPMD.
#
# reference semantics:
#   feat = concat(node_feats[src], node_feats[dst], axis=1)      # [E, 2048]
#   h    = relu(feat @ W1 + b1)                                  # [E, 1024]
#   out  = h @ W2 + b2                                           # [E, 1024]
#
# Sharding: edges split evenly across 8 cores (8192 each); node table and
# weights replicated.  All matmul operands are bf16 (host-cast): same PE rate
# as fp32r (1 row/cycle) but half the DMA/DVE traffic, 1.0 instead of 1.5
# cycles/row transposes, and rel-err ~3e-3 (vs the 2e-2 gate).
#
# Dataflow (per 512-edge supertile, 16 per core): layer 1 is computed
# TRANSPOSED (out1T[h, e] = sum_k W1[k, hc].T @ featT[k, e]) so the relu
# output lands directly in the [h, e] layout that layer 2 needs as its
# stationary operand -- this kills the 8 h-transposes per 128-edge tile and
# the hT PSUM->SBUF copy of the old layout.  Relu+bias runs on the Scalar
# engine with per-partition bias (b1 rearranged so partition p of chunk hc
# holds b1[hc*128+p]), reading PSUM and writing bf16 SBUF in one pass.
#
# Per supertile:
#   - 8 indirect-DMA gathers (4 subtiles x src/dst), [128, 1024] bf16 rows
#   - 64 PE transposes -> featT chunks [128 f, 512 e] (staged via PSUM,
#     copied to SBUF by DVE in [128, 1024] chunks)
#   - L1: 8 h-chunks x 16 k-chunks matmuls (ap=512) -> psum [128 h, 512 e]
#   - ACT relu + per-partition b1, psum -> hT SBUF bf16
#   - L2: 4 e-chunks x 16 matmuls (ap=512) -> psum [128 e, 1024]
#   - DVE + b2 -> out tile f32 -> HWDGE store
import os
import sys

import numpy as np

for _p in ("/opt/trn_rl_repo",):
    if _p not in sys.path:
        sys.path.insert(0, _p)

N_NODES = 50000
D_NODE = 1024
D_HID = 1024
N_CORES = 8
E_TOTAL = 65536
E_CORE = E_TOTAL // N_CORES
P = 128


def build_nc(e_core=E_CORE, n_nodes=N_NODES):
    import concourse.bass as bass
    import concourse.mybir as mybir
    import concourse.tile as tile
    from concourse import bacc
    from concourse.masks import make_identity

    f32 = mybir.dt.float32
    bf16 = mybir.dt.bfloat16
    i32 = mybir.dt.int32

    T = e_core // P  # 64 subtiles of 128 edges
    SUP = 4  # subtiles per supertile
    S = T // SUP  # 16 supertiles
    ES = SUP * P  # 512 edges per supertile
    KD = (2 * D_NODE) // P  # 16 contraction chunks, layer 1
    KH = D_HID // P  # 8 contraction chunks, layer 2

    nc = bacc.Bacc(None, target_bir_lowering=False)

    nf = nc.dram_tensor("node_feats", [n_nodes, D_NODE], bf16, kind="ExternalInput")
    # W1/W2 arrive host-permuted as [128, K, 1024] (partition-major: entry
    # [p, k, :] = W[k*128+p, :]) so each weight load is one 32KB descriptor
    # per partition -- the hardware DGE queues are descriptor-dispatch-rate
    # limited (~60GB/s with 2KB descriptors), which used to stall layer 1 of
    # supertile 0 until ~50us.  b1 likewise arrives as [128, KH].
    w1 = nc.dram_tensor("W1", [P, KD, D_HID], bf16, kind="ExternalInput")
    w2 = nc.dram_tensor("W2", [P, KH, D_HID], bf16, kind="ExternalInput")
    b1 = nc.dram_tensor("b1", [P, KH], f32, kind="ExternalInput")
    b2 = nc.dram_tensor("b2", [D_HID], f32, kind="ExternalInput")
    src = nc.dram_tensor("src", [e_core], i32, kind="ExternalInput")
    dst = nc.dram_tensor("dst", [e_core], i32, kind="ExternalInput")
    out = nc.dram_tensor("out", [e_core, D_HID], f32, kind="ExternalOutput")

    nf_ap = nf.ap()
    # edge e of the core shard maps to (p, t) = (e // T, e % T): index loads
    # and output stores are contiguous per partition.
    out_v = out.ap().rearrange("(p t) h -> p t h", t=T)

    with tile.TileContext(nc) as tc:
        with (
            tc.tile_pool(name="const", bufs=1) as const_pool,
            tc.tile_pool(name="wpool", bufs=1) as wpool,
            tc.tile_pool(name="gather", bufs=2) as gather_pool,
            tc.tile_pool(name="featT", bufs=2) as featT_pool,
            tc.tile_pool(name="hT", bufs=2) as hT_pool,
            tc.tile_pool(name="outp", bufs=2) as out_pool,
            tc.tile_pool(name="pstf", bufs=2, space="PSUM") as psT_pool,
            tc.tile_pool(name="ps1", bufs=3, space="PSUM") as ps1_pool,
            tc.tile_pool(name="ps2", bufs=2, space="PSUM") as ps2_pool,
        ):
            # ---- constants / weights ----
            # zeros tile first: PE warmup transposes read it, and a single
            # gpsimd memset is ready ~1us before the identity construction.
            zeros_bf = const_pool.tile([P, P], bf16)
            nc.gpsimd.memset(zeros_bf[:], 0.0)
            ident_f32 = const_pool.tile([P, P], f32)
            ident = const_pool.tile([P, P], bf16)

            def build_ident():
                make_identity(nc, ident_f32[:])
                nc.vector.tensor_copy(ident[:], ident_f32[:])

            # src and dst indices side by side in one tile so a whole
            # supertile (4 subtiles x src+dst = 1024 rows) gathers with a
            # single SWDGE instruction (offset AP [128, 2, 4]) instead of 8 --
            # the ~1.1us/instruction descriptor-generation cost on the gpsimd
            # sequencer was the startup critical path.
            idx_all = const_pool.tile([P, 2, T], i32)
            nc.sync.dma_start(idx_all[:, 0, :], src.ap().rearrange("(p t) -> p t", t=T))
            nc.sync.dma_start(idx_all[:, 1, :], dst.ap().rearrange("(p t) -> p t", t=T))

            # W1 as [128, KD, 1024]: chunk k holds rows k*128..k*128+127 (f on
            # partitions -> natural lhsT for the transposed layer-1 matmuls).
            # W2 as [128, KH, 1024] likewise (h on partitions -> natural
            # moving operand for layer 2).  Interleaved 2:1 so W2 arrives
            # before supertile 0 reaches layer 2, and spread across three DGE
            # queues so descriptor generation isn't serialized behind one
            # sequencer at startup.
            # Strict priority: W1 first (it gates layer 1 of supertile 0;
            # the hardware DMA queues drain in order), split across both
            # HWDGE queues; then b1T (gates the first relu and with it the
            # ps1 pool), then W2 (needed ~15us later), then the 512KB b2
            # broadcast.
            w1_sb = wpool.tile([P, KD, D_HID], bf16)
            w2_sb = wpool.tile([P, KH, D_HID], bf16)
            nc.sync.dma_start(w1_sb[:, : KD // 2], w1.ap()[:, : KD // 2])
            nc.scalar.dma_start(w1_sb[:, KD // 2 :], w1.ap()[:, KD // 2 :])
            b1T = const_pool.tile([P, KH], f32)
            nc.sync.dma_start(b1T[:], b1.ap())
            nc.sync.dma_start(w2_sb[:, : KH // 2], w2.ap()[:, : KH // 2])
            nc.scalar.dma_start(w2_sb[:, KH // 2 :], w2.ap()[:, KH // 2 :])
            b2_bc = const_pool.tile([P, D_HID], f32)
            nc.scalar.dma_start(b2_bc[:], b2.ap()[None, :].to_broadcast([P, D_HID]))

            gsync = const_pool.tile([P, 1], bf16)

            def stage_G(s, after=None):
                """Indirect gathers for supertile s: gf[p, i*4+j, :] =
                node_feats[idx_all[p, i, 4s+j], :] (i = 0 src / 1 dst).
                All 4 src rows gather before the dst rows -- the transposes
                consume src chunks (pairs 0..3) first.

                `after`: featT tile whose last chunk must be written before
                these gathers start.  Engine queues run in program order, so
                this dummy gpsimd read is the only way to hold gathers back;
                used for supertiles 1-2, whose 2MB would otherwise compete
                with the W1 stream that gates L1(0) on startup HBM bandwidth.
                """
                if after is not None:
                    nc.gpsimd.tensor_copy(gsync[:], after[:, KD - 1, 0:1])
                gf = gather_pool.tile([P, 2 * SUP, D_NODE], bf16, tag="gf")
                for i in range(2):
                    for j in range(SUP):
                        t = SUP * s + j
                        nc.gpsimd.indirect_dma_start(
                            out=gf[:, i * SUP + j, :],
                            out_offset=None,
                            in_=nf_ap[:],
                            in_offset=bass.IndirectOffsetOnAxis(
                                ap=idx_all[:, i, t : t + 1], axis=0
                            ),
                        )
                return gf

            def warmup(n):
                """Dummy PE transposes of a zeros tile into scratch PSUM:
                keeps the Tensor-engine clock at full p-state while real
                operands are still in flight (a PE idle gap drops the clock
                to 1.2 GHz for the next ~3us of work)."""
                wps = psT_pool.tile([P, 2, ES], bf16, tag="psT")
                for i in range(n):
                    nc.tensor.transpose(wps[:, i % 2, 0:P], zeros_bf[:], zeros_bf[:])

            def stage_T(s, gf, fill=0):
                """PE transposes -> featT [128, KD, ES] bf16 in SBUF."""
                fT = featT_pool.tile([P, KD, ES], bf16, tag="featT")
                for pair in range(KD // 2):
                    if fill and pair == KD // 4:
                        warmup(fill)  # src gathers done, dst still landing
                    psT = psT_pool.tile([P, 2, ES], bf16, tag="psT")
                    for q in range(2):
                        k = 2 * pair + q
                        i, kk = (0, k) if k < KD // 2 else (1, k - KD // 2)
                        for j in range(SUP):
                            blk = gf[:, i * SUP + j, kk * P : (kk + 1) * P]
                            nc.tensor.transpose(
                                psT[:, q, j * P : (j + 1) * P], blk, ident[:]
                            )
                    nc.vector.tensor_copy(fT[:, 2 * pair : 2 * pair + 2, :], psT[:])
                return fT

            def stage_L1(s, fT):
                """Transposed layer 1 + fused relu/bias -> hT [128, KH, ES]."""
                hT = hT_pool.tile([P, KH, ES], bf16, tag="hT")
                for hc in range(KH):
                    ps1 = ps1_pool.tile([P, ES], f32, tag="ps1")
                    for k in range(KD):
                        nc.tensor.matmul(
                            ps1[:],
                            w1_sb[:, k, hc * P : (hc + 1) * P],
                            fT[:, k, :],
                            start=(k == 0),
                            stop=(k == KD - 1),
                        )
                    nc.scalar.activation(
                        hT[:, hc, :],
                        ps1[:],
                        mybir.ActivationFunctionType.Relu,
                        bias=b1T[:, hc : hc + 1],
                    )
                return hT

            def stage_L2(s, hT):
                """Layer 2 per 128-edge subtile, +b2; one batched store per
                supertile (partition p's 4 output rows are contiguous 16KB in
                DRAM, so the store is one descriptor per partition instead of
                4x128 4KB descriptors -- the store queue was within ~10%% of
                its descriptor-dispatch limit)."""
                o_sb = out_pool.tile([P, SUP, D_HID], f32, tag="osb")
                for ec in range(SUP):
                    for half in range(2):
                        ps2 = ps2_pool.tile([P, 512], f32, tag="ps2")
                        for k in range(KH):
                            nc.tensor.matmul(
                                ps2[:],
                                hT[:, k, ec * P : (ec + 1) * P],
                                w2_sb[:, k, half * 512 : (half + 1) * 512],
                                start=(k == 0),
                                stop=(k == KH - 1),
                            )
                        nc.vector.tensor_add(
                            o_sb[:, ec, half * 512 : (half + 1) * 512],
                            ps2[:],
                            b2_bc[:, half * 512 : (half + 1) * 512],
                        )
                    # store in 2-subtile chunks (8KB contiguous per partition);
                    # the last supertile stores per subtile so the kernel tail
                    # only waits on a 512KB transfer instead of 2MB.
                    step = 1 if s == S - 1 else 2
                    if s == S - 1 and ec == SUP - 1:
                        for half in range(2):
                            nc.scalar.dma_start(
                                out_v[:, SUP * s + ec, half * 512 : (half + 1) * 512],
                                o_sb[:, ec, half * 512 : (half + 1) * 512],
                            )
                    elif (ec + 1) % step == 0:
                        lo = ec + 1 - step
                        nc.scalar.dma_start(
                            out_v[:, SUP * s + lo : SUP * s + ec + 1, :],
                            o_sb[:, lo : ec + 1, :],
                        )

            # PE warmup: the Tensor engine clock ramps (0.65 -> 1.2 -> 2.4 GHz)
            # only after ~3us of continuous busy, and the first real transposes
            # can't start until the first gathers land (~13us in).  Keep the PE
            # busy from t~1us with dummy identity transposes into a scratch
            # PSUM tile so the clock is at full speed when real work arrives.
            # software pipeline: gathers run 2 supertiles ahead, transposes 1
            # ahead; PE stream per iteration is [L1(s) | T(s+1) | L2(s)] so the
            # last relu of s and the featT copies of s+1 hide under PE work.
            # warmup(200) spans the ~10us from the framework preamble until
            # supertile 0's src gathers land.  G(1) is issued only after T(0):
            # startup is HBM-bandwidth-bound, and G(1)'s 2MB of gathers would
            # otherwise compete with the W1 stream that gates L1(0) (T(1)
            # doesn't need G(1) until ~55us).
            gt = {0: stage_G(0)}
            build_ident()
            warmup(200)
            # Supertile 0 is special-cased: its src gathers land ~7us before
            # the dst gathers, so the src-chunk halves of the first three L1
            # hc groups (3 open ps1 accumulation groups) run between the src
            # and dst transposes instead of idling the PE on the dst wait.
            gf0 = gt.pop(0)
            fT0 = featT_pool.tile([P, KD, ES], bf16, tag="featT")

            def t0_pairs(lo, hi):
                for pair in range(lo, hi):
                    psT = psT_pool.tile([P, 2, ES], bf16, tag="psT")
                    for q in range(2):
                        k = 2 * pair + q
                        i, kk = (0, k) if k < KD // 2 else (1, k - KD // 2)
                        for j in range(SUP):
                            blk = gf0[:, i * SUP + j, kk * P : (kk + 1) * P]
                            nc.tensor.transpose(
                                psT[:, q, j * P : (j + 1) * P], blk, ident[:]
                            )
                    nc.vector.tensor_copy(
                        fT0[:, 2 * pair : 2 * pair + 2, :], psT[:]
                    )

            ps1_0 = {}

            def l1_0_half(hc, k_lo, k_hi):
                if hc not in ps1_0:
                    ps1_0[hc] = ps1_pool.tile(
                        [P, ES], f32, tag="ps1", name=f"ps1_s0_{hc}"
                    )
                for k in range(k_lo, k_hi):
                    nc.tensor.matmul(
                        ps1_0[hc][:],
                        w1_sb[:, k, hc * P : (hc + 1) * P],
                        fT0[:, k, :],
                        start=(k == 0),
                        stop=(k == KD - 1),
                    )

            hT0 = hT_pool.tile([P, KH, ES], bf16, tag="hT")
            t0_pairs(0, KD // 4)  # src chunk transposes
            for hc in range(3):
                l1_0_half(hc, 0, KD // 2)  # src halves, groups stay open
            t0_pairs(KD // 4, KD // 2)  # dst chunk transposes
            for hc in range(KH):
                if hc < 3:
                    l1_0_half(hc, KD // 2, KD)  # close the open groups
                else:
                    l1_0_half(hc, 0, KD)
                nc.scalar.activation(
                    hT0[:, hc, :],
                    ps1_0[hc][:],
                    mybir.ActivationFunctionType.Relu,
                    bias=b1T[:, hc : hc + 1],
                )
            fTs = {0: fT0}
            if S > 1:
                gt[1] = stage_G(1, after=fTs[0])
            for s in range(S):
                hT = hT0 if s == 0 else stage_L1(s, fTs.pop(s))
                fTs.pop(s, None)
                if s + 1 < S:
                    fTs[s + 1] = stage_T(s + 1, gt.pop(s + 1))
                if s + 2 < S:
                    gt[s + 2] = stage_G(
                        s + 2, after=fTs[s + 1] if s + 2 == 2 else None
                    )
                stage_L2(s, hT)

    nc.compile()
    return nc


LAST_RESULTS = None


def kernel(**inputs):
    global LAST_RESULTS
    import ml_dtypes
    from concourse.bass_utils import run_bass_kernel_spmd

    bf16 = ml_dtypes.bfloat16
    node_feats = np.ascontiguousarray(np.asarray(inputs["node_feats"]).astype(bf16))
    # weights/b1 pre-permuted to the partition-major DRAM layouts build_nc
    # declares (one large descriptor per partition on load)
    W1 = np.ascontiguousarray(
        np.asarray(inputs["W1"]).astype(bf16).reshape(16, 128, 1024).swapaxes(0, 1)
    )
    W2 = np.ascontiguousarray(
        np.asarray(inputs["W2"]).astype(bf16).reshape(8, 128, 1024).swapaxes(0, 1)
    )
    b1 = np.ascontiguousarray(
        np.asarray(inputs["b1"], np.float32).reshape(8, 128).T
    )
    b2 = np.ascontiguousarray(np.asarray(inputs["b2"], np.float32))
    src = np.ascontiguousarray(np.asarray(inputs["src"]).astype(np.int32))
    dst = np.ascontiguousarray(np.asarray(inputs["dst"]).astype(np.int32))

    nc = build_nc()

    in_maps = []
    for c in range(N_CORES):
        sl = slice(c * E_CORE, (c + 1) * E_CORE)
        in_maps.append(
            {
                "node_feats": node_feats,
                "W1": W1,
                "W2": W2,
                "b1": b1,
                "b2": b2,
                "src": src[sl],
                "dst": dst[sl],
            }
        )

    trace = bool(int(os.environ.get("KERNEL_TRACE", "0")))
    kw = {}
    if trace and bool(int(os.environ.get("KERNEL_TRACE_ALL", "0"))):
        kw["trace_cores"] = list(range(N_CORES))
    res = run_bass_kernel_spmd(
        nc, in_maps, core_ids=list(range(N_CORES)), trace=trace, **kw
    )
    LAST_RESULTS = res
    return np.concatenate([r["out"] for r in res.results], axis=0)
